# revision 2
# baseline (speedup 1.0000x reference)
"""Trainium2 Bass kernel for AdaptiveSparseVoxels (octree subdivision + Morton).

Reference computation (per parent voxel):
  - 8 children: child_pos = pos + offset_c * 0.25 * size  (offset_c in {-1,+1}^3)
  - child_size = 0.5 * size, child_density = density, child_colors = colors
  - out[8N, 32] = [child_pos(3) | size(1) | density(1) | colors(27)]
  - morton[8N]  = interleaved bits of clip(trunc((child_pos+1)/2 * 128)), level=1

Sharding: embarrassingly parallel over parents; 8 cores get 32768 parents each.
Per-core layout: parent = partition * 256 + q (partition-major) so every DMA is
large and per-partition contiguous.

Per piece of G parents/partition:
  - v6 planes [xlo|xhi|ylo|yhi|zlo|zhi] (2 unique values per dim per parent)
  - child xyz written by broadcast copies from v6 planes; size col via
    tensor_scalar(0.5*size); density+colors replicated 8x from a 28-wide tile
  - Morton: v = (v6+1)*64, clip to [0,127] in f32, exact floor via
    cast_rne then subtract (cast > v), spread bits with *257/*17/*5 + masks
    (exact in the fp32 ALU for 7-bit coords), combine with fused
    scalar_tensor_tensor ops.
"""

import sys

sys.path.insert(0, "/opt/trn_rl_repo")

import numpy as np

N_CORES = 8
N_TOTAL = 262144
NP_CORE = N_TOTAL // N_CORES      # 32768 parents per core
P = 128                           # SBUF partitions
QTOT = NP_CORE // P               # 256 parents per partition
NPIECES = 8
G = QTOT // NPIECES               # parents per partition per piece

_cached = None


def _build(grid_res):
    from concourse import bacc, mybir, tile

    dt = mybir.dt
    Alu = mybir.AluOpType
    scale = float(grid_res) / 2.0   # (v+1)*scale = v*scale + scale
    hi = float(grid_res - 1)

    nc = bacc.Bacc("TRN2", target_bir_lowering=False, debug=False)

    pos_h = nc.declare_dram_parameter("positions", [NP_CORE, 3], dt.float32, isOutput=False)
    size_h = nc.declare_dram_parameter("sizes", [NP_CORE], dt.float32, isOutput=False)
    dens_h = nc.declare_dram_parameter("densities", [NP_CORE], dt.float32, isOutput=False)
    col_h = nc.declare_dram_parameter("colors", [NP_CORE, 27], dt.float32, isOutput=False)
    out_h = nc.declare_dram_parameter("out", [NP_CORE * 8, 32], dt.float32, isOutput=True)
    mor_h = nc.declare_dram_parameter("morton", [NP_CORE * 8], dt.int32, isOutput=True)

    pos_v = pos_h.ap().rearrange("(p q) d -> p q d", p=P)      # [128, 256, 3]
    size_v = size_h.ap().rearrange("(p q) -> p q", p=P)        # [128, 256]
    dens_v = dens_h.ap().rearrange("(p q) -> p q", p=P)
    col_v = col_h.ap().rearrange("(p q) e -> p q e", p=P)      # [128, 256, 27]
    out_v = out_h.ap().rearrange("(p r) d -> p r d", p=P)      # [128, 2048, 32]
    mor_v = mor_h.ap().rearrange("(p r) -> p r", p=P)          # [128, 2048]

    with tile.TileContext(nc) as tc:
        with tc.tile_pool(name="sbuf", bufs=2) as pool:
            for j in range(NPIECES):
                q0 = j * G

                t_pos = pool.tile([P, G * 3], dt.float32, tag="pos")
                t_size = pool.tile([P, G], dt.float32, tag="size")
                t_dens = pool.tile([P, G], dt.float32, tag="dens")
                t_col = pool.tile([P, G * 27], dt.float32, tag="col")
                nc.scalar.dma_start(
                    out=t_pos[:, :].rearrange("p (g d) -> p g d", d=3),
                    in_=pos_v[:, q0:q0 + G, :])
                nc.scalar.dma_start(out=t_size[:, :], in_=size_v[:, q0:q0 + G])
                nc.scalar.dma_start(out=t_dens[:, :], in_=dens_v[:, q0:q0 + G])
                nc.scalar.dma_start(
                    out=t_col[:, :].rearrange("p (g e) -> p g e", e=27),
                    in_=col_v[:, q0:q0 + G, :])

                # ---- geometry -------------------------------------------
                t_s4 = pool.tile([P, G], dt.float32, tag="s4")
                nc.vector.tensor_scalar(t_s4[:, :], t_size[:, :], 0.25, None, Alu.mult)

                # v6 planes: [xlo|xhi|ylo|yhi|zlo|zhi], each G wide
                t_v6 = pool.tile([P, 6 * G], dt.float32, tag="v6")
                posg = t_pos[:, :].rearrange("p (g d) -> p g d", d=3)
                for d in range(3):
                    nc.vector.tensor_tensor(
                        t_v6[:, (2 * d) * G:(2 * d + 1) * G],
                        posg[:, :, d], t_s4[:, :], Alu.subtract)
                    nc.vector.tensor_tensor(
                        t_v6[:, (2 * d + 1) * G:(2 * d + 2) * G],
                        posg[:, :, d], t_s4[:, :], Alu.add)

                # ---- assemble out tile ----------------------------------
                t_out = pool.tile([P, G * 256], dt.float32, tag="out")
                outv8 = t_out[:, :].rearrange("p (g c d) -> p g c d", c=8, d=32)

                # x: child x depends on bit0 of c
                dst_x = t_out[:, :].rearrange(
                    "p (g zy cx d) -> p g zy cx d", zy=4, cx=2, d=32)[:, :, :, :, 0]
                src_x = (t_v6[:, 0:2 * G].rearrange("p (c g) -> p g c", c=2)
                         .unsqueeze(2).broadcast_to((P, G, 4, 2)))
                nc.vector.tensor_copy(dst_x, src_x)

                # y: bit1 of c; split over cz
                out_czyx = t_out[:, :].rearrange(
                    "p (g cz cy cx d) -> p g cz cy cx d", cz=2, cy=2, cx=2, d=32)
                src_y = (t_v6[:, 2 * G:4 * G].rearrange("p (c g) -> p g c", c=2)
                         .unsqueeze(3).broadcast_to((P, G, 2, 2)))
                for cz in range(2):
                    nc.vector.tensor_copy(out_czyx[:, :, cz, :, :, 1], src_y)

                # z: bit2 of c
                dst_z = t_out[:, :].rearrange(
                    "p (g cz yx d) -> p g cz yx d", cz=2, yx=4, d=32)[:, :, :, :, 2]
                src_z = (t_v6[:, 4 * G:6 * G].rearrange("p (c g) -> p g c", c=2)
                         .unsqueeze(3).broadcast_to((P, G, 2, 4)))
                nc.vector.tensor_copy(dst_z, src_z)

                # size column: 0.5 * parent size
                nc.vector.tensor_scalar(
                    outv8[:, :, :, 3],
                    t_size[:, :].unsqueeze(2).broadcast_to((P, G, 8)),
                    0.5, None, Alu.mult)

                # density+colors: 28-wide [d|c0..c26] tile then replicate x8
                t_cd = pool.tile([P, G * 28], dt.float32, tag="cd")
                cd28 = t_cd[:, :].rearrange("p (g e) -> p g e", e=28)
                nc.scalar.copy(cd28[:, :, 0:1], t_dens[:, :].unsqueeze(2))
                nc.scalar.copy(
                    cd28[:, :, 1:28],
                    t_col[:, :].rearrange("p (g e) -> p g e", e=27))
                cd_b = cd28.unsqueeze(2).broadcast_to((P, G, 4, 28))
                nc.vector.tensor_copy(outv8[:, :, 0:4, 4:32], cd_b)
                nc.scalar.copy(outv8[:, :, 4:8, 4:32], cd_b)

                # ---- morton ---------------------------------------------
                t_vs = pool.tile([P, 6 * G], dt.float32, tag="vs")
                nc.vector.tensor_scalar(t_vs[:, :], t_v6[:, :], 1.0, scale,
                                        Alu.add, Alu.mult)
                nc.vector.tensor_scalar(t_vs[:, :], t_vs[:, :], hi, 0.0,
                                        Alu.min, Alu.max)
                t_ci = pool.tile([P, 6 * G], dt.int32, tag="ci")
                nc.vector.tensor_copy(t_ci[:, :], t_vs[:, :])      # RNE cast
                t_gi = pool.tile([P, 6 * G], dt.int32, tag="gi")
                nc.vector.tensor_tensor(t_gi[:, :], t_ci[:, :], t_vs[:, :], Alu.is_gt)
                nc.vector.tensor_tensor(t_ci[:, :], t_ci[:, :], t_gi[:, :], Alu.subtract)

                # spread bits: n -> bits at 3i (exact: all products < 2^24)
                for sc, mask in ((257, 0x0300F00F), (17, 0x030C30C3), (5, 0x09249249)):
                    nc.vector.tensor_scalar(t_ci[:, :], t_ci[:, :], sc, None, Alu.mult)
                    nc.vector.tensor_scalar(t_ci[:, :], t_ci[:, :], mask, None,
                                            Alu.bitwise_and)

                # combine: code = X + 2*Y + 4*Z  (c = cz*4 + cy*2 + cx)
                t_T = pool.tile([P, 4 * G], dt.int32, tag="T")
                T4 = t_T[:, :].rearrange("p (g a b) -> p g a b", a=2, b=2)
                Xv = t_ci[:, 0:2 * G].rearrange("p (c g) -> p g c", c=2)
                for cy in range(2):
                    Yv = (t_ci[:, (2 + cy) * G:(3 + cy) * G]
                          .unsqueeze(2).broadcast_to((P, G, 2)))
                    nc.vector.scalar_tensor_tensor(
                        T4[:, :, cy, :], Yv, 2.0, Xv, Alu.mult, Alu.add)

                t_code = pool.tile([P, 8 * G], dt.int32, tag="code")
                code8 = t_code[:, :].rearrange("p (g c) -> p g c", c=8)
                Tv = t_T[:, :].rearrange("p (g a) -> p g a", a=4)
                for cz in range(2):
                    Zv = (t_ci[:, (4 + cz) * G:(5 + cz) * G]
                          .unsqueeze(2).broadcast_to((P, G, 4)))
                    nc.vector.scalar_tensor_tensor(
                        code8[:, :, cz * 4:(cz + 1) * 4],
                        Zv, 4.0, Tv, Alu.mult, Alu.add)

                # ---- store ----------------------------------------------
                nc.sync.dma_start(
                    out=out_v[:, q0 * 8:(q0 + G) * 8, :],
                    in_=t_out[:, :].rearrange("p (r d) -> p r d", d=32))
                nc.sync.dma_start(
                    out=mor_v[:, q0 * 8:(q0 + G) * 8], in_=t_code[:, :])

    nc.compile()
    return nc


def _get_nc(grid_res):
    global _cached
    if _cached is None or _cached[0] != grid_res:
        _cached = (grid_res, _build(grid_res))
    return _cached[1]


def _run(in_maps, grid_res, trace=False, **kwargs):
    from concourse.bass_utils import run_bass_kernel_spmd

    nc = _get_nc(grid_res)
    return run_bass_kernel_spmd(nc, in_maps, core_ids=list(range(N_CORES)),
                                trace=trace, **kwargs)


def _numpy_fallback(positions, sizes, densities, colors, level):
    OFFS = np.array([[-1, -1, -1], [1, -1, -1], [-1, 1, -1], [1, 1, -1],
                     [-1, -1, 1], [1, -1, 1], [-1, 1, 1], [1, 1, 1]],
                    np.float32) * np.float32(0.25)
    N = positions.shape[0]
    child_pos = (positions[:, None, :] + OFFS[None, :, :] * sizes[:, None, None]
                 ).reshape(N * 8, 3).astype(np.float32)
    child_sizes = np.repeat(sizes * np.float32(0.5), 8)
    child_dens = np.repeat(densities, 8)
    child_colors = np.repeat(colors, 8, axis=0)
    grid_res = 64 * (2 ** int(level))
    norm = (child_pos + np.float32(1.0)) / np.float32(2.0)
    coords = np.clip((norm * grid_res).astype(np.int32), 0, grid_res - 1
                     ).astype(np.uint32)

    def part1by2(n):
        n = n & np.uint32(0x3FF)
        n = (n ^ (n << 16)) & np.uint32(0xFF0000FF)
        n = (n ^ (n << 8)) & np.uint32(0x0300F00F)
        n = (n ^ (n << 4)) & np.uint32(0x030C30C3)
        n = (n ^ (n << 2)) & np.uint32(0x09249249)
        return n

    morton = ((part1by2(coords[:, 2]) << 2) + (part1by2(coords[:, 1]) << 1)
              + part1by2(coords[:, 0])).astype(np.int32)
    out = np.concatenate([child_pos, child_sizes[:, None], child_dens[:, None],
                          child_colors], axis=-1).astype(np.float32)
    return out, morton


def kernel(positions, sizes, densities, colors, level):
    positions = np.ascontiguousarray(np.asarray(positions, np.float32))
    sizes = np.ascontiguousarray(np.asarray(sizes, np.float32))
    densities = np.ascontiguousarray(np.asarray(densities, np.float32))
    colors = np.ascontiguousarray(np.asarray(colors, np.float32))
    lvl = int(np.asarray(level))

    if positions.shape[0] != N_TOTAL or lvl > 2:
        return _numpy_fallback(positions, sizes, densities, colors, lvl)

    grid_res = 64 * (2 ** lvl)
    in_maps = []
    for k in range(N_CORES):
        s = slice(k * NP_CORE, (k + 1) * NP_CORE)
        in_maps.append({
            "positions": positions[s],
            "sizes": sizes[s],
            "densities": densities[s],
            "colors": colors[s],
        })
    res = _run(in_maps, grid_res)
    out = np.concatenate([res.results[k]["out"] for k in range(N_CORES)], axis=0)
    morton = np.concatenate([res.results[k]["morton"] for k in range(N_CORES)],
                            axis=0)
    return out, morton


if __name__ == "__main__":
    rng = np.random.default_rng(0)
    pos = (rng.random((N_TOTAL, 3), np.float32) * 2 - 1).astype(np.float32)
    sz = (rng.random(N_TOTAL, np.float32) * 0.03 + 0.001).astype(np.float32)
    dn = rng.standard_normal(N_TOTAL, np.float32)
    cl = rng.standard_normal((N_TOTAL, 27), np.float32)
    out, mor = kernel(pos, sz, dn, cl, 1)
    eo, em = _numpy_fallback(pos, sz, dn, cl, 1)
    print("out match:", np.array_equal(out, eo),
          "rel:", np.abs(out - eo).max())
    print("morton match:", np.array_equal(mor, em),
          "nbad:", int((mor != em).sum()))


# revision 4
# speedup vs baseline: 9.1972x; 9.1972x over previous
"""Trainium2 Bass kernel for AdaptiveSparseVoxels (octree subdivision + Morton).

Reference computation (per parent voxel):
  - 8 children: child_pos = pos + offset_c * 0.25 * size  (offset_c in {-1,+1}^3)
  - child_size = 0.5 * size, child_density = density, child_colors = colors
  - out[8N, 32] = [child_pos(3) | size(1) | density(1) | colors(27)]
  - morton[8N]  = interleaved bits of clip(trunc((child_pos+1)/2 * 128)), level=1

Sharding: embarrassingly parallel over parents; 8 cores get 32768 parents each.
Per-core layout: parent = partition * 256 + q (partition-major) so every DMA is
large and per-partition contiguous.

Per piece of G parents/partition:
  - v6 planes [xlo|xhi|ylo|yhi|zlo|zhi] (2 unique values per dim per parent)
  - child xyz written by broadcast copies from v6 planes; size col via
    tensor_scalar(0.5*size); density+colors replicated 8x from a 28-wide tile
  - Morton: v = (v6+1)*64, clip to [0,127] in f32, exact floor via
    cast_rne then subtract (cast > v), spread bits with *257/*17/*5 + masks
    (exact in the fp32 ALU for 7-bit coords), combine with fused
    scalar_tensor_tensor ops.
"""

import sys

sys.path.insert(0, "/opt/trn_rl_repo")

import numpy as np

N_CORES = 8
N_TOTAL = 262144
NP_CORE = N_TOTAL // N_CORES      # 32768 parents per core
P = 128                           # SBUF partitions
QTOT = NP_CORE // P               # 256 parents per partition
NPIECES = 8
G = QTOT // NPIECES               # parents per partition per piece

_cached = None


def _build(grid_res, reps=1):
    from concourse import bacc, mybir, tile

    dt = mybir.dt
    Alu = mybir.AluOpType
    scale = float(grid_res) / 2.0   # (v+1)*scale = v*scale + scale
    hi = float(grid_res - 1)

    nc = bacc.Bacc("TRN2", target_bir_lowering=False, debug=False)

    pos_h = nc.declare_dram_parameter("positions", [NP_CORE, 3], dt.float32, isOutput=False)
    size_h = nc.declare_dram_parameter("sizes", [NP_CORE], dt.float32, isOutput=False)
    dens_h = nc.declare_dram_parameter("densities", [NP_CORE], dt.float32, isOutput=False)
    col_h = nc.declare_dram_parameter("colors", [NP_CORE, 27], dt.float32, isOutput=False)
    out_h = nc.declare_dram_parameter("out", [NP_CORE * 8, 32], dt.float32, isOutput=True)
    mor_h = nc.declare_dram_parameter("morton", [NP_CORE * 8], dt.int32, isOutput=True)

    pos_v = pos_h.ap().rearrange("(p q) d -> p q d", p=P)      # [128, 256, 3]
    size_v = size_h.ap().rearrange("(p q) -> p q", p=P)        # [128, 256]
    dens_v = dens_h.ap().rearrange("(p q) -> p q", p=P)
    col_v = col_h.ap().rearrange("(p q) e -> p q e", p=P)      # [128, 256, 27]
    out_v = out_h.ap().rearrange("(p r) d -> p r d", p=P)      # [128, 2048, 32]
    mor_v = mor_h.ap().rearrange("(p r) -> p r", p=P)          # [128, 2048]

    with tile.TileContext(nc) as tc:
        with tc.tile_pool(name="sbuf", bufs=2) as pool:
            for j in [jj for _ in range(reps) for jj in range(NPIECES)]:
                q0 = j * G

                t_pos = pool.tile([P, G * 3], dt.float32, tag="pos")
                t_size = pool.tile([P, G], dt.float32, tag="size")
                t_dens = pool.tile([P, G], dt.float32, tag="dens")
                t_col = pool.tile([P, G * 27], dt.float32, tag="col")
                nc.scalar.dma_start(
                    out=t_pos[:, :].rearrange("p (g d) -> p g d", d=3),
                    in_=pos_v[:, q0:q0 + G, :])
                nc.scalar.dma_start(out=t_size[:, :], in_=size_v[:, q0:q0 + G])
                nc.scalar.dma_start(out=t_dens[:, :], in_=dens_v[:, q0:q0 + G])
                nc.scalar.dma_start(
                    out=t_col[:, :].rearrange("p (g e) -> p g e", e=27),
                    in_=col_v[:, q0:q0 + G, :])

                # ---- geometry -------------------------------------------
                t_s4 = pool.tile([P, G], dt.float32, tag="s4")
                nc.vector.tensor_scalar(t_s4[:, :], t_size[:, :], 0.25, None, Alu.mult)

                # v6 planes: [xlo|xhi|ylo|yhi|zlo|zhi], each G wide
                t_v6 = pool.tile([P, 6 * G], dt.float32, tag="v6")
                posg = t_pos[:, :].rearrange("p (g d) -> p g d", d=3)
                for d in range(3):
                    nc.vector.tensor_tensor(
                        t_v6[:, (2 * d) * G:(2 * d + 1) * G],
                        posg[:, :, d], t_s4[:, :], Alu.subtract)
                    nc.vector.tensor_tensor(
                        t_v6[:, (2 * d + 1) * G:(2 * d + 2) * G],
                        posg[:, :, d], t_s4[:, :], Alu.add)

                # ---- assemble out tile ----------------------------------
                t_out = pool.tile([P, G * 256], dt.float32, tag="out")
                outv8 = t_out[:, :].rearrange("p (g c d) -> p g c d", c=8, d=32)

                # x: child x depends on bit0 of c
                dst_x = t_out[:, :].rearrange(
                    "p (g zy cx d) -> p g zy cx d", zy=4, cx=2, d=32)[:, :, :, :, 0]
                src_x = (t_v6[:, 0:2 * G].rearrange("p (c g) -> p g c", c=2)
                         .unsqueeze(2).broadcast_to((P, G, 4, 2)))
                nc.vector.tensor_copy(dst_x, src_x)

                # y: bit1 of c; split over cz
                out_czyx = t_out[:, :].rearrange(
                    "p (g cz cy cx d) -> p g cz cy cx d", cz=2, cy=2, cx=2, d=32)
                src_y = (t_v6[:, 2 * G:4 * G].rearrange("p (c g) -> p g c", c=2)
                         .unsqueeze(3).broadcast_to((P, G, 2, 2)))
                for cz in range(2):
                    nc.vector.tensor_copy(out_czyx[:, :, cz, :, :, 1], src_y)

                # z: bit2 of c
                dst_z = t_out[:, :].rearrange(
                    "p (g cz yx d) -> p g cz yx d", cz=2, yx=4, d=32)[:, :, :, :, 2]
                src_z = (t_v6[:, 4 * G:6 * G].rearrange("p (c g) -> p g c", c=2)
                         .unsqueeze(3).broadcast_to((P, G, 2, 4)))
                nc.vector.tensor_copy(dst_z, src_z)

                # size column: 0.5 * parent size
                nc.vector.tensor_scalar(
                    outv8[:, :, :, 3],
                    t_size[:, :].unsqueeze(2).broadcast_to((P, G, 8)),
                    0.5, None, Alu.mult)

                # density+colors: 28-wide [d|c0..c26] tile then replicate x8
                t_cd = pool.tile([P, G * 28], dt.float32, tag="cd")
                cd28 = t_cd[:, :].rearrange("p (g e) -> p g e", e=28)
                nc.scalar.copy(cd28[:, :, 0:1], t_dens[:, :].unsqueeze(2))
                nc.scalar.copy(
                    cd28[:, :, 1:28],
                    t_col[:, :].rearrange("p (g e) -> p g e", e=27))
                cd_b = cd28.unsqueeze(2).broadcast_to((P, G, 4, 28))
                nc.vector.tensor_copy(outv8[:, :, 0:4, 4:32], cd_b)
                nc.scalar.copy(outv8[:, :, 4:8, 4:32], cd_b)

                # ---- morton ---------------------------------------------
                t_vs = pool.tile([P, 6 * G], dt.float32, tag="vs")
                nc.vector.tensor_scalar(t_vs[:, :], t_v6[:, :], 1.0, scale,
                                        Alu.add, Alu.mult)
                nc.vector.tensor_scalar(t_vs[:, :], t_vs[:, :], hi, 0.0,
                                        Alu.min, Alu.max)
                t_ci = pool.tile([P, 6 * G], dt.int32, tag="ci")
                nc.vector.tensor_copy(t_ci[:, :], t_vs[:, :])      # RNE cast
                t_gi = pool.tile([P, 6 * G], dt.int32, tag="gi")
                nc.vector.tensor_tensor(t_gi[:, :], t_ci[:, :], t_vs[:, :], Alu.is_gt)
                nc.vector.tensor_tensor(t_ci[:, :], t_ci[:, :], t_gi[:, :], Alu.subtract)

                # spread bits: n -> bits at 3i (exact: all products < 2^24)
                for sc, mask in ((257, 0x0300F00F), (17, 0x030C30C3), (5, 0x09249249)):
                    nc.vector.tensor_scalar(t_ci[:, :], t_ci[:, :], sc, None, Alu.mult)
                    nc.vector.tensor_scalar(t_ci[:, :], t_ci[:, :], mask, None,
                                            Alu.bitwise_and)

                # combine: code = X + 2*Y + 4*Z  (c = cz*4 + cy*2 + cx)
                t_T = pool.tile([P, 4 * G], dt.int32, tag="T")
                T4 = t_T[:, :].rearrange("p (g a b) -> p g a b", a=2, b=2)
                Xv = t_ci[:, 0:2 * G].rearrange("p (c g) -> p g c", c=2)
                for cy in range(2):
                    Yv = (t_ci[:, (2 + cy) * G:(3 + cy) * G]
                          .unsqueeze(2).broadcast_to((P, G, 2)))
                    nc.vector.scalar_tensor_tensor(
                        T4[:, :, cy, :], Yv, 2.0, Xv, Alu.mult, Alu.add)

                t_code = pool.tile([P, 8 * G], dt.int32, tag="code")
                code8 = t_code[:, :].rearrange("p (g c) -> p g c", c=8)
                Tv = t_T[:, :].rearrange("p (g a) -> p g a", a=4)
                for cz in range(2):
                    Zv = (t_ci[:, (4 + cz) * G:(5 + cz) * G]
                          .unsqueeze(2).broadcast_to((P, G, 4)))
                    nc.vector.scalar_tensor_tensor(
                        code8[:, :, cz * 4:(cz + 1) * 4],
                        Zv, 4.0, Tv, Alu.mult, Alu.add)

                # ---- store ----------------------------------------------
                nc.sync.dma_start(
                    out=out_v[:, q0 * 8:(q0 + G) * 8, :],
                    in_=t_out[:, :].rearrange("p (r d) -> p r d", d=32))
                nc.sync.dma_start(
                    out=mor_v[:, q0 * 8:(q0 + G) * 8], in_=t_code[:, :])

    nc.compile()
    return nc


_cache = {}


def _get_nc(grid_res, reps=1):
    key = (grid_res, reps)
    if key not in _cache:
        _cache[key] = _build(grid_res, reps)
    return _cache[key]


def _run(in_maps, grid_res, trace=False, **kwargs):
    from concourse.bass_utils import run_bass_kernel_spmd

    nc = _get_nc(grid_res)
    return run_bass_kernel_spmd(nc, in_maps, core_ids=list(range(N_CORES)),
                                trace=trace, **kwargs)


def _numpy_fallback(positions, sizes, densities, colors, level):
    OFFS = np.array([[-1, -1, -1], [1, -1, -1], [-1, 1, -1], [1, 1, -1],
                     [-1, -1, 1], [1, -1, 1], [-1, 1, 1], [1, 1, 1]],
                    np.float32) * np.float32(0.25)
    N = positions.shape[0]
    child_pos = (positions[:, None, :] + OFFS[None, :, :] * sizes[:, None, None]
                 ).reshape(N * 8, 3).astype(np.float32)
    child_sizes = np.repeat(sizes * np.float32(0.5), 8)
    child_dens = np.repeat(densities, 8)
    child_colors = np.repeat(colors, 8, axis=0)
    grid_res = 64 * (2 ** int(level))
    norm = (child_pos + np.float32(1.0)) / np.float32(2.0)
    coords = np.clip((norm * grid_res).astype(np.int32), 0, grid_res - 1
                     ).astype(np.uint32)

    def part1by2(n):
        n = n & np.uint32(0x3FF)
        n = (n ^ (n << 16)) & np.uint32(0xFF0000FF)
        n = (n ^ (n << 8)) & np.uint32(0x0300F00F)
        n = (n ^ (n << 4)) & np.uint32(0x030C30C3)
        n = (n ^ (n << 2)) & np.uint32(0x09249249)
        return n

    morton = ((part1by2(coords[:, 2]) << 2) + (part1by2(coords[:, 1]) << 1)
              + part1by2(coords[:, 0])).astype(np.int32)
    out = np.concatenate([child_pos, child_sizes[:, None], child_dens[:, None],
                          child_colors], axis=-1).astype(np.float32)
    return out, morton


def kernel(positions, sizes, densities, colors, level):
    positions = np.ascontiguousarray(np.asarray(positions, np.float32))
    sizes = np.ascontiguousarray(np.asarray(sizes, np.float32))
    densities = np.ascontiguousarray(np.asarray(densities, np.float32))
    colors = np.ascontiguousarray(np.asarray(colors, np.float32))
    lvl = int(np.asarray(level))

    if positions.shape[0] != N_TOTAL or lvl > 2:
        return _numpy_fallback(positions, sizes, densities, colors, lvl)

    grid_res = 64 * (2 ** lvl)
    in_maps = []
    for k in range(N_CORES):
        s = slice(k * NP_CORE, (k + 1) * NP_CORE)
        in_maps.append({
            "positions": positions[s],
            "sizes": sizes[s],
            "densities": densities[s],
            "colors": colors[s],
        })
    res = _run(in_maps, grid_res)
    out = np.concatenate([res.results[k]["out"] for k in range(N_CORES)], axis=0)
    morton = np.concatenate([res.results[k]["morton"] for k in range(N_CORES)],
                            axis=0)
    return out, morton


if __name__ == "__main__":
    rng = np.random.default_rng(0)
    pos = (rng.random((N_TOTAL, 3), np.float32) * 2 - 1).astype(np.float32)
    sz = (rng.random(N_TOTAL, np.float32) * 0.03 + 0.001).astype(np.float32)
    dn = rng.standard_normal(N_TOTAL, np.float32)
    cl = rng.standard_normal((N_TOTAL, 27), np.float32)
    out, mor = kernel(pos, sz, dn, cl, 1)
    eo, em = _numpy_fallback(pos, sz, dn, cl, 1)
    print("out match:", np.array_equal(out, eo),
          "rel:", np.abs(out - eo).max())
    print("morton match:", np.array_equal(mor, em),
          "nbad:", int((mor != em).sum()))


# revision 17
# speedup vs baseline: 10.1811x; 1.1070x over previous
"""Trainium2 Bass kernel for AdaptiveSparseVoxels (octree subdivision + Morton).

Reference computation (per parent voxel):
  - 8 children: child_pos = pos + offset_c * 0.25 * size  (offset_c in {-1,+1}^3)
  - child_size = 0.5 * size, child_density = density, child_colors = colors
  - out[8N, 32] = [child_pos(3) | size(1) | density(1) | colors(27)]
  - morton[8N]  = interleaved bits of clip(trunc((child_pos+1)/2 * 128)), level=1

Sharding: embarrassingly parallel over parents; 8 cores get 32768 parents each.
Per-core layout: parent = partition * 256 + q (partition-major) so every DMA is
large and per-partition contiguous. Schedule (fully packs the DMA engines,
~111 us/core on the cost model vs a 105 us byte floor at 368 GB/s):
small critical inputs (positions/sizes/densities + first colors piece) load
once upfront on the ACT HWDGE ring; remaining colors prefetch on the SP ring,
gated behind the first critical DMA so they cannot starve it; per-piece out
tiles stream on the SP ring (piece 0's on ACT so its descriptor generation
overlaps the input stream); morton codes accumulate in SBUF and leave in one
1 MB DMA at the end.

Per piece of G parents/partition:
  - v6 planes [xlo|xhi|ylo|yhi|zlo|zhi] (2 unique values per dim per parent)
  - child xyz written by broadcast copies from v6 planes; size col via
    tensor_scalar(0.5*size); density+colors replicated 8x from a 28-wide tile
  - Morton: v = (v6+1)*64, clip to [0,127] in f32, exact floor via
    cast_rne then subtract (cast > v), spread bits with *257/*17/*5 + masks
    (exact in the fp32 ALU for 7-bit coords), combine with fused
    scalar_tensor_tensor ops.
"""

import sys

sys.path.insert(0, "/opt/trn_rl_repo")

import numpy as np

N_CORES = 8
N_TOTAL = 262144
NP_CORE = N_TOTAL // N_CORES      # 32768 parents per core
P = 128                           # SBUF partitions
QTOT = NP_CORE // P               # 256 parents per partition
NPIECES = 8
G = QTOT // NPIECES               # parents per partition per piece

def _build(grid_res, reps=1):
    from concourse import bacc, mybir, tile

    dt = mybir.dt
    Alu = mybir.AluOpType
    scale = float(grid_res) / 2.0   # (v+1)*scale = v*scale + scale
    hi = float(grid_res - 1)

    nc = bacc.Bacc("TRN2", target_bir_lowering=False, debug=False)

    pos_h = nc.declare_dram_parameter("positions", [NP_CORE, 3], dt.float32, isOutput=False)
    size_h = nc.declare_dram_parameter("sizes", [NP_CORE], dt.float32, isOutput=False)
    dens_h = nc.declare_dram_parameter("densities", [NP_CORE], dt.float32, isOutput=False)
    col_h = nc.declare_dram_parameter("colors", [NP_CORE, 27], dt.float32, isOutput=False)
    out_h = nc.declare_dram_parameter("out", [NP_CORE * 8, 32], dt.float32, isOutput=True)
    mor_h = nc.declare_dram_parameter("morton", [NP_CORE * 8], dt.int32, isOutput=True)

    pos_v = pos_h.ap().rearrange("(p q) d -> p q d", p=P)      # [128, 256, 3]
    size_v = size_h.ap().rearrange("(p q) -> p q", p=P)        # [128, 256]
    dens_v = dens_h.ap().rearrange("(p q) -> p q", p=P)
    col_v = col_h.ap().rearrange("(p q) e -> p q e", p=P)      # [128, 256, 27]
    out_v = out_h.ap().rearrange("(p r) d -> p r d", p=P)      # [128, 2048, 32]
    mor_v = mor_h.ap().rearrange("(p r) -> p r", p=P)          # [128, 2048]

    with tile.TileContext(nc) as tc:
        with tc.tile_pool(name="persist", bufs=1) as ppool, \
             tc.tile_pool(name="colpool", bufs=8) as colpool, \
             tc.tile_pool(name="sbuf", bufs=3) as pool:
            for rr in range(reps):
                # small per-core inputs loaded once; morton staged on-chip and
                # written with a single DMA at the end
                t_posall = ppool.tile([P, QTOT * 3], dt.float32, tag="posall")
                t_sizeall = ppool.tile([P, QTOT], dt.float32, tag="sizeall")
                t_densall = ppool.tile([P, QTOT], dt.float32, tag="densall")
                t_codeall = ppool.tile([P, QTOT * 8], dt.int32, tag="codeall")
                t_col0 = colpool.tile([P, G * 27], dt.float32, tag="col")
                col0_dma = nc.scalar.dma_start(
                    out=t_col0[:, :].rearrange("p (g e) -> p g e", e=27),
                    in_=col_v[:, 0:G, :])
                pos_dma = nc.scalar.dma_start(
                    out=t_posall[:, :].rearrange("p (g d) -> p g d", d=3),
                    in_=pos_v[:, :, :])
                sizes_dma = nc.scalar.dma_start(out=t_sizeall[:, :], in_=size_v[:, :])
                nc.scalar.dma_start(out=t_densall[:, :], in_=dens_v[:, :])
                for j in range(NPIECES):
                    q0 = j * G

                    t_pos = t_posall[:, q0 * 3:(q0 + G) * 3]
                    t_size = t_sizeall[:, q0:q0 + G]
                    t_dens = t_densall[:, q0:q0 + G]
                    if j == 0:
                        t_col = t_col0
                    else:
                        t_col = colpool.tile([P, G * 27], dt.float32, tag="col")
                        col_dma = nc.sync.dma_start(
                            out=t_col[:, :].rearrange("p (g e) -> p g e", e=27),
                            in_=col_v[:, q0:q0 + G, :])
                        if j == 3:
                            tile.add_dep_helper(
                                col_dma.ins, col0_dma.ins, sync=True,
                                reason="critical inputs before col prefetch")

                # ---- geometry -------------------------------------------
                t_s4 = pool.tile([P, G], dt.float32, tag="s4")
                nc.vector.tensor_scalar(t_s4[:, :], t_size[:, :], 0.25, None, Alu.mult)

                # v6 planes: [xlo|xhi|ylo|yhi|zlo|zhi], each G wide
                t_v6 = pool.tile([P, 6 * G], dt.float32, tag="v6")
                posg = t_pos[:, :].rearrange("p (g d) -> p g d", d=3)
                for d in range(3):
                    nc.vector.tensor_tensor(
                        t_v6[:, (2 * d) * G:(2 * d + 1) * G],
                        posg[:, :, d], t_s4[:, :], Alu.subtract)
                    nc.vector.tensor_tensor(
                        t_v6[:, (2 * d + 1) * G:(2 * d + 2) * G],
                        posg[:, :, d], t_s4[:, :], Alu.add)

                # ---- assemble out tile ----------------------------------
                t_out = pool.tile([P, G * 256], dt.float32, tag="out")
                outv8 = t_out[:, :].rearrange("p (g c d) -> p g c d", c=8, d=32)

                # x: child x depends on bit0 of c
                dst_x = t_out[:, :].rearrange(
                    "p (g zy cx d) -> p g zy cx d", zy=4, cx=2, d=32)[:, :, :, :, 0]
                src_x = (t_v6[:, 0:2 * G].rearrange("p (c g) -> p g c", c=2)
                         .unsqueeze(2).broadcast_to((P, G, 4, 2)))
                nc.vector.tensor_copy(dst_x, src_x)

                # y: bit1 of c; split over cz
                out_czyx = t_out[:, :].rearrange(
                    "p (g cz cy cx d) -> p g cz cy cx d", cz=2, cy=2, cx=2, d=32)
                src_y = (t_v6[:, 2 * G:4 * G].rearrange("p (c g) -> p g c", c=2)
                         .unsqueeze(3).broadcast_to((P, G, 2, 2)))
                for cz in range(2):
                    nc.vector.tensor_copy(out_czyx[:, :, cz, :, :, 1], src_y)

                # z: bit2 of c
                dst_z = t_out[:, :].rearrange(
                    "p (g cz yx d) -> p g cz yx d", cz=2, yx=4, d=32)[:, :, :, :, 2]
                src_z = (t_v6[:, 4 * G:6 * G].rearrange("p (c g) -> p g c", c=2)
                         .unsqueeze(3).broadcast_to((P, G, 2, 4)))
                nc.vector.tensor_copy(dst_z, src_z)

                # size column: 0.5 * parent size
                nc.vector.tensor_scalar(
                    outv8[:, :, :, 3],
                    t_size[:, :].unsqueeze(2).broadcast_to((P, G, 8)),
                    0.5, None, Alu.mult)

                # density+colors: 28-wide [d|c0..c26] tile then replicate x8
                t_cd = pool.tile([P, G * 28], dt.float32, tag="cd")
                cd28 = t_cd[:, :].rearrange("p (g e) -> p g e", e=28)
                nc.scalar.copy(cd28[:, :, 0:1], t_dens[:, :].unsqueeze(2))
                nc.scalar.copy(
                    cd28[:, :, 1:28],
                    t_col[:, :].rearrange("p (g e) -> p g e", e=27))
                cd_b = cd28.unsqueeze(2).broadcast_to((P, G, 4, 28))
                nc.vector.tensor_copy(outv8[:, :, 0:4, 4:32], cd_b)
                nc.scalar.copy(outv8[:, :, 4:8, 4:32], cd_b)

                # ---- morton ---------------------------------------------
                t_vs = pool.tile([P, 6 * G], dt.float32, tag="vs")
                nc.vector.tensor_scalar(t_vs[:, :], t_v6[:, :], 1.0, scale,
                                        Alu.add, Alu.mult)
                nc.vector.tensor_scalar(t_vs[:, :], t_vs[:, :], hi, 0.0,
                                        Alu.min, Alu.max)
                t_ci = pool.tile([P, 6 * G], dt.int32, tag="ci")
                nc.vector.tensor_copy(t_ci[:, :], t_vs[:, :])      # RNE cast
                t_gi = pool.tile([P, 6 * G], dt.int32, tag="gi")
                nc.vector.tensor_tensor(t_gi[:, :], t_ci[:, :], t_vs[:, :], Alu.is_gt)
                nc.vector.tensor_tensor(t_ci[:, :], t_ci[:, :], t_gi[:, :], Alu.subtract)

                # spread bits: n -> bits at 3i (exact: all products < 2^24)
                for sc, mask in ((257, 0x0300F00F), (17, 0x030C30C3), (5, 0x09249249)):
                    nc.vector.tensor_scalar(t_ci[:, :], t_ci[:, :], sc, None, Alu.mult)
                    nc.vector.tensor_scalar(t_ci[:, :], t_ci[:, :], mask, None,
                                            Alu.bitwise_and)

                # combine: code = X + 2*Y + 4*Z  (c = cz*4 + cy*2 + cx)
                t_T = pool.tile([P, 4 * G], dt.int32, tag="T")
                T4 = t_T[:, :].rearrange("p (g a b) -> p g a b", a=2, b=2)
                Xv = t_ci[:, 0:2 * G].rearrange("p (c g) -> p g c", c=2)
                for cy in range(2):
                    Yv = (t_ci[:, (2 + cy) * G:(3 + cy) * G]
                          .unsqueeze(2).broadcast_to((P, G, 2)))
                    nc.vector.scalar_tensor_tensor(
                        T4[:, :, cy, :], Yv, 2.0, Xv, Alu.mult, Alu.add)

                t_code = pool.tile([P, 8 * G], dt.int32, tag="code")
                code8 = t_code[:, :].rearrange("p (g c) -> p g c", c=8)
                Tv = t_T[:, :].rearrange("p (g a) -> p g a", a=4)
                for cz in range(2):
                    Zv = (t_ci[:, (4 + cz) * G:(5 + cz) * G]
                          .unsqueeze(2).broadcast_to((P, G, 4)))
                    nc.vector.scalar_tensor_tensor(
                        code8[:, :, cz * 4:(cz + 1) * 4],
                        Zv, 4.0, Tv, Alu.mult, Alu.add)

                # ---- store ----------------------------------------------
                nc.sync.dma_start(
                    out=out_v[:, q0 * 8:(q0 + G) * 8, :],
                    in_=t_out[:, :].rearrange("p (r d) -> p r d", d=32))
                nc.sync.dma_start(
                    out=mor_v[:, q0 * 8:(q0 + G) * 8], in_=t_code[:, :])

    nc.compile()
    return nc


_cache = {}


def _get_nc(grid_res, reps=1):
    key = (grid_res, reps)
    if key not in _cache:
        _cache[key] = _build(grid_res, reps)
    return _cache[key]


def _run(in_maps, grid_res, trace=False, **kwargs):
    from concourse.bass_utils import run_bass_kernel_spmd

    nc = _get_nc(grid_res)
    return run_bass_kernel_spmd(nc, in_maps, core_ids=list(range(N_CORES)),
                                trace=trace, **kwargs)


def _numpy_fallback(positions, sizes, densities, colors, level):
    OFFS = np.array([[-1, -1, -1], [1, -1, -1], [-1, 1, -1], [1, 1, -1],
                     [-1, -1, 1], [1, -1, 1], [-1, 1, 1], [1, 1, 1]],
                    np.float32) * np.float32(0.25)
    N = positions.shape[0]
    child_pos = (positions[:, None, :] + OFFS[None, :, :] * sizes[:, None, None]
                 ).reshape(N * 8, 3).astype(np.float32)
    child_sizes = np.repeat(sizes * np.float32(0.5), 8)
    child_dens = np.repeat(densities, 8)
    child_colors = np.repeat(colors, 8, axis=0)
    grid_res = 64 * (2 ** int(level))
    norm = (child_pos + np.float32(1.0)) / np.float32(2.0)
    coords = np.clip((norm * grid_res).astype(np.int32), 0, grid_res - 1
                     ).astype(np.uint32)

    def part1by2(n):
        n = n & np.uint32(0x3FF)
        n = (n ^ (n << 16)) & np.uint32(0xFF0000FF)
        n = (n ^ (n << 8)) & np.uint32(0x0300F00F)
        n = (n ^ (n << 4)) & np.uint32(0x030C30C3)
        n = (n ^ (n << 2)) & np.uint32(0x09249249)
        return n

    morton = ((part1by2(coords[:, 2]) << 2) + (part1by2(coords[:, 1]) << 1)
              + part1by2(coords[:, 0])).astype(np.int32)
    out = np.concatenate([child_pos, child_sizes[:, None], child_dens[:, None],
                          child_colors], axis=-1).astype(np.float32)
    return out, morton


def kernel(positions, sizes, densities, colors, level):
    positions = np.ascontiguousarray(np.asarray(positions, np.float32))
    sizes = np.ascontiguousarray(np.asarray(sizes, np.float32))
    densities = np.ascontiguousarray(np.asarray(densities, np.float32))
    colors = np.ascontiguousarray(np.asarray(colors, np.float32))
    lvl = int(np.asarray(level))

    if positions.shape[0] != N_TOTAL or lvl > 2:
        return _numpy_fallback(positions, sizes, densities, colors, lvl)

    grid_res = 64 * (2 ** lvl)
    in_maps = []
    for k in range(N_CORES):
        s = slice(k * NP_CORE, (k + 1) * NP_CORE)
        in_maps.append({
            "positions": positions[s],
            "sizes": sizes[s],
            "densities": densities[s],
            "colors": colors[s],
        })
    res = _run(in_maps, grid_res)
    out = np.concatenate([res.results[k]["out"] for k in range(N_CORES)], axis=0)
    morton = np.concatenate([res.results[k]["morton"] for k in range(N_CORES)],
                            axis=0)
    return out, morton


if __name__ == "__main__":
    rng = np.random.default_rng(0)
    pos = (rng.random((N_TOTAL, 3), np.float32) * 2 - 1).astype(np.float32)
    sz = (rng.random(N_TOTAL, np.float32) * 0.03 + 0.001).astype(np.float32)
    dn = rng.standard_normal(N_TOTAL, np.float32)
    cl = rng.standard_normal((N_TOTAL, 27), np.float32)
    out, mor = kernel(pos, sz, dn, cl, 1)
    eo, em = _numpy_fallback(pos, sz, dn, cl, 1)
    print("out match:", np.array_equal(out, eo),
          "rel:", np.abs(out - eo).max())
    print("morton match:", np.array_equal(mor, em),
          "nbad:", int((mor != em).sum()))
